# revision 2
# baseline (speedup 1.0000x reference)
"""3D Haar DWT (clean-mode subband stack) on 8 Trainium2 NeuronCores.

Problem (hardcoded): inputs (4, 128, 128, 128, 4) f32, A (128, 128) f32 Haar
analysis operator. Output (4, 64, 64, 64, 32) f32 = 8 subbands stacked on the
channel axis (LLL, LLH, LHL, LHH, HLL, HLH, HHL, HHH) x 4 channels.

Sharding: pure data parallel over (batch, d1-half): core k handles
b = k // 2, d1 range [64*(k%2), 64*(k%2)+64). The Haar transform is a 2-tap
non-overlapping filter (rows of A touch only columns 2i, 2i+1), so splitting
d1 on an even boundary requires no communication.

The kernel is memory-bound (HBM ~358 GB/s per core), so the whole on-device
datapath runs in bf16: the host converts the input slab to bf16 (8 MiB/core
instead of 16) and the output is stored as bf16 (8 MiB instead of 16), halving
HBM traffic vs f32. bf16 keeps ~3 decimal digits; the absmax-relative error
lands ~1e-3, far inside the 2e-2 gate.

Per-core pipeline (slab pre-transposed on host to [d2, d1, d3, c] bf16):
  1. DMA in 1 MiB chunks (8 d1 slices), partitions = d2, 8 KiB descriptors.
  2. d3 butterfly then d1 butterfly on DVE (free-axis ops, bf16 = 2x rate).
  3. d2 transform as PE matmul with a single stationary matrix 0.5*sign(A^T)
     (exact in bf16, loaded once — no weight reloads).
  4. PSUM -> SBUF evacuation on ACT applies the residual 1/sqrt(2) and casts
     to bf16; doubles as the subband split.
  5. DMA out per (s1, s3) block on SWDGE (so stores never head-of-line-block
     the load queue); host reassembles the subband-major layout in f32.

Scale bookkeeping: reference applies A three times (factor s = 1/sqrt(2) per
nonzero). Here the d3/d1 butterflies apply +/-1, the matmul 0.5, and the
evacuation 1/sqrt(2): each path gets 0.5/sqrt(2) = s^3 — exactly the
reference scaling.
"""

import sys

import numpy as np

if "/opt/trn_rl_repo" not in sys.path:
    sys.path.insert(0, "/opt/trn_rl_repo")

B, N, C = 4, 128, 4
N_CORES = 8
SLAB = 64          # d1 extent per core
D1C = 8            # d1 values per chunk
NCHUNK = SLAB // D1C
PAIRS = D1C // 2   # d1 pairs per chunk

_BASS_CACHE = {}

# Stationary weights are the exactly-representable +-0.5 sign pattern of A^T;
# the residual 1/sqrt(2) is applied in the PSUM evacuation.
EVAC_SCALE = float(1.0 / np.sqrt(2.0))


def _haar_matrix():
    s = np.float32(1.0 / np.sqrt(2.0))
    A = np.zeros((N, N), dtype=np.float32)
    for i in range(N // 2):
        A[i, 2 * i] = s
        A[i, 2 * i + 1] = s
        A[64 + i, 2 * i] = -s
        A[64 + i, 2 * i + 1] = s
    return A


def _reference_numpy(inputs, A):
    # Fallback only: exact reference math on host (used if A is not Haar).
    x = np.einsum("ij,bpjqc->bpiqc", A, inputs)
    x = np.einsum("ij,bjpqc->bipqc", A, x)
    x = np.einsum("ij,bpqjc->bpqic", A, x)
    m = x.shape[1] // 2
    subs = [
        x[:, :m, :m, :m, :], x[:, :m, :m, m:, :],
        x[:, :m, m:, :m, :], x[:, :m, m:, m:, :],
        x[:, m:, :m, :m, :], x[:, m:, :m, m:, :],
        x[:, m:, m:, :m, :], x[:, m:, m:, m:, :],
    ]
    return np.concatenate(subs, axis=-1).astype(np.float32)


def _build_bass():
    import concourse.bacc as bacc
    import concourse.mybir as mybir
    import concourse.tile as tile

    f32 = mybir.dt.float32
    bf16 = mybir.dt.bfloat16

    # Bacc (not raw Bass): its compile() pipeline splits multi-sem waits into
    # EventSemaphore instructions — TRN2 instructions have one wait slot.
    nc = bacc.Bacc("TRN2", target_bir_lowering=False, debug=False)
    # x is host-pre-transposed to [d2, d1, d3, c] bf16 so each load descriptor
    # covers an 8 KiB contiguous run per partition.
    x = nc.dram_tensor("x", [N, SLAB, N, C], bf16, kind="ExternalInput")
    atp = nc.dram_tensor("atp", [N, N], bf16, kind="ExternalInput")
    # y dims: (s1, s3, i2, o1, o3, c); i2 = s2*64 + o2. i2 outermost of the
    # spatial dims so each store descriptor is a 2 KiB contiguous run.
    y = nc.dram_tensor("y", [2, 2, N, 32, 64, C], bf16, kind="ExternalOutput")

    with tile.TileContext(nc) as tc:
        with (
            tc.tile_pool(name="const", bufs=1) as cpool,
            tc.tile_pool(name="io", bufs=4) as tpool,
            tc.tile_pool(name="mid", bufs=2) as mpool,
            tc.tile_pool(name="psum", bufs=4, space="PSUM") as ppool,
        ):
            atp_sb = cpool.tile([N, N], bf16)

            for ci in range(NCHUNK):
                # 1. load chunk: [d2 | d1_local, d3*c] — one 1 MiB DMA,
                # 128 descriptors of 8 KiB.
                T = tpool.tile([N, D1C, N * C], bf16, tag="T")
                nc.sync.dma_start(
                    out=T[:],
                    in_=x[:, ci * D1C:(ci + 1) * D1C].rearrange("p a q c -> p a (q c)"),
                )
                if ci == 0:
                    # consts after the first bulk load so the data pipeline
                    # starts immediately
                    nc.sync.dma_start(out=atp_sb[:], in_=atp[:, :])

                # view: p, d1, o3, d3-parity, c
                Tv = T[:].rearrange("p a (m t c) -> p a m t c", t=2, c=C)

                # 2a. d3 butterfly: U[:, :, 0] = even+odd (lo), [:, :, 1] =
                # odd-even (hi); layout (p, d1, s3, o3, c).
                U = mpool.tile([N, D1C, 2, 64, C], bf16, tag="U")
                nc.vector.tensor_add(
                    out=U[:, :, 0], in0=Tv[:, :, :, 0], in1=Tv[:, :, :, 1]
                )
                nc.vector.tensor_sub(
                    out=U[:, :, 1], in0=Tv[:, :, :, 1], in1=Tv[:, :, :, 0]
                )

                # 2b. d1 butterfly across pair members; V layout
                # (p, s1, pair, s3, o3, c).
                V = mpool.tile([N, 2, PAIRS, 2, 64, C], bf16, tag="V")
                Up = U[:].rearrange("p (q m) k o c -> p q m (k o c)", m=2)
                Vv = V[:].rearrange("p s q k o c -> p s q (k o c)")
                nc.vector.tensor_add(out=Vv[:, 0], in0=Up[:, :, 0], in1=Up[:, :, 1])
                nc.vector.tensor_sub(out=Vv[:, 1], in0=Up[:, :, 1], in1=Up[:, :, 0])

                # staging: (p, s1, s3, pair, o3*c)
                Yst = mpool.tile([N, 2, 2, PAIRS, 64 * C], bf16, tag="Yst")

                for pp in range(PAIRS):
                    # 3. d2 transform: one stationary 0.5*sign(A^T) for all
                    # matmuls; each output is one PSUM bank (512 f32).
                    ps = ppool.tile([N, 2, 512], f32, tag="ps")
                    nc.tensor.matmul(
                        ps[:, 0], lhsT=atp_sb[:], rhs=Vv[:, 0, pp],
                        start=True, stop=True,
                    )
                    nc.tensor.matmul(
                        ps[:, 1], lhsT=atp_sb[:], rhs=Vv[:, 1, pp],
                        start=True, stop=True,
                    )
                    # 4. evacuate + subband split + residual scale + bf16 cast
                    # on the scalar engine.
                    nc.scalar.mul(
                        Yst[:, :, :, pp],
                        ps[:].rearrange("p s (k f) -> p s k f", k=2),
                        EVAC_SCALE,
                    )

                # 5. store per (s1, s3): y[s1, s3, :, o1 range] <- [i2 | o1,
                # o3*c] on SWDGE (gpsimd) so stores never head-of-line-block
                # the load queue on the SP sequencer.
                for s1 in range(2):
                    for s3 in range(2):
                        nc.gpsimd.dma_start(
                            out=y[s1, s3, :, ci * PAIRS:(ci + 1) * PAIRS].rearrange(
                                "p a q c -> p a (q c)"
                            ),
                            in_=Yst[:, s1, s3],
                        )
    nc.compile()
    return nc


def _prepare(x, A):
    """Host-side prep shared with test.py: build (nc, in_maps)."""
    import ml_dtypes

    if "nc" not in _BASS_CACHE:
        _BASS_CACHE["nc"] = _build_bass()
    nc = _BASS_CACHE["nc"]

    atp = np.ascontiguousarray(
        (0.5 * np.sign(A.T)).astype(ml_dtypes.bfloat16)
    )
    xb = x.astype(ml_dtypes.bfloat16)
    in_maps = []
    for k in range(N_CORES):
        b, h = divmod(k, 2)
        # pre-transpose slab to [d2, d1, d3, c] for contiguous load rows
        in_maps.append(
            {
                "x": np.ascontiguousarray(
                    xb[b, h * SLAB:(h + 1) * SLAB].transpose(1, 0, 2, 3)
                ),
                "atp": atp,
            }
        )
    return nc, in_maps


def _assemble(results):
    """Gather per-core bf16 y tensors into the full f32 output."""
    out = np.empty((B, 64, 64, 64, 8 * C), np.float32)
    for k in range(N_CORES):
        b, h = divmod(k, 2)
        arr = results[k]["y"].astype(np.float32).reshape(2, 2, 2, 64, 32, 64, C)
        # (s1, s3, s2, o2, o1, o3, c) -> (o1, o2, o3, s1, s2, s3, c)
        out[b, 32 * h:32 * h + 32] = (
            arr.transpose(4, 3, 5, 0, 2, 1, 6).reshape(32, 64, 64, 8 * C)
        )
    return out


def kernel(**inputs):
    x = np.ascontiguousarray(np.asarray(inputs["inputs"], dtype=np.float32))
    A = np.asarray(inputs["A"], dtype=np.float32)
    assert x.shape == (B, N, N, N, C), x.shape

    if not np.allclose(A, _haar_matrix(), atol=1e-5):
        # Kernel hardcodes the 2-tap Haar structure; fall back for generic A.
        return _reference_numpy(x, A)

    from concourse.bass_utils import run_bass_kernel_spmd

    nc, in_maps = _prepare(x, A)
    res = run_bass_kernel_spmd(nc, in_maps, core_ids=list(range(N_CORES)))
    return _assemble(res.results)
